# revision 42
# baseline (speedup 1.0000x reference)
"""Multi-head self-attention (b=2, n=2048, emb=1024, heads=16) on 8 trn2 cores.

Sharding: core c = (b, hg) with b = c // 4, hg = c % 4. Data parallel over
batch, tensor parallel over head-groups (4 heads / 256 emb-cols per core).
Each core computes Q/K/V projections for its heads, full attention for its
heads, and a partial output projection ctx_hg @ Wo[:, hg_slice].T of shape
[2048, 1024]. The host sums the 4 partials per batch (Megatron row-parallel
reduce done on host) and adds the rank-1 bias term bv @ Wo.T + bo.

Device layout notes:
- x^T [emb, n] fp16 (4MB = 32KB/partition) is loaded ONCE and stays resident
  in SBUF; both head-pairs' projections read it, so there is no mid-kernel
  x DMA traffic and no DMA wait can head-of-line-block the in-order PE queue.
- DMA issue is split across the Sync and GpSimd queues so sequencer DGE time
  doesn't serialize the startup (weights on Sync, x + output stores on
  GpSimd).
- Q^T, K^T are produced in [dq, n] layout (dq = head-major), V in natural
  [n, dv] layout augmented with a ones column per head -> the ctx matmul
  ctxT[65, nq] = V_aug^T @ E^T produces softmax row-sums in row 64 for free.
- Attention runs on 512-wide nq chunks: S matmuls, 1024-wide exps straight
  out of PSUM, and ctx matmuls all stream 512 columns, halving the PE
  instruction count (and its stall overhead) vs 256-wide chunks.
- Softmax normalization: row-sum staged to partition 0 (DVE), broadcast
  (GpSimd), reciprocal_approx_fast (single custom-DVE op, ~5x faster than
  InstReciprocal; denominators are strictly positive sums of exps so the
  approx's edge cases can't occur), then one fused multiply straight out of
  the ctx PSUM bank into ctxT fp16.
- Output projection accumulates both head-pair contributions into one PSUM
  group (2 matmuls back-to-back), needing a single PSUM->SBUF copy and no
  SBUF accumulator array.
- All matmuls run in float16 (1 cyc/col on PE; 10-bit mantissa keeps the
  overall error ~7e-4 scale-relative, validated vs fp32).
- q/k biases are added on-device (fused into the PSUM->SBUF copy);
  v/o biases are exactly the rank-1 host-side term above.
"""

import os
import sys

for _p in ("/opt/trn_rl_repo", "/root/.axon_site/_ro/trn_rl_repo"):
    if os.path.isdir(_p) and _p not in sys.path:
        sys.path.append(_p)

import numpy as np

import concourse.bass as bass  # noqa: F401  (engine types pulled via nc)
import concourse.mybir as mybir
import concourse.tile as tile
from concourse import bacc
from concourse.bass_utils import run_bass_kernel_spmd

B, N, EMB, HEADS, HD = 2, 2048, 1024, 16, 64
N_CORES = 8
TP = 4                      # head-group shards per batch
DQ = EMB // TP              # 256 emb-cols (4 heads) per core
SCALE = HD ** -0.5          # 0.125

F32 = mybir.dt.float32
F16 = mybir.dt.float16
FP = mybir.ActivationFunctionType

NQ = 512                    # nq chunk for projections / out-proj
NJ = N // NQ                # 4 nq chunks
NQA = 512                   # nq chunk for attention
NJA = N // NQA              # 4 attention nq chunks
NKC = 128                   # nk chunk (ctx contraction)
NT = N // NKC               # 16 nk chunks
KC = EMB // 128             # 8 e chunks
GK = 2                      # nk chunks per S-psum group (1024-wide exps)
NG = NT // GK               # 8 groups per (head, j)


def build_program():
    """Build + compile the single SPMD program all 8 cores run."""
    nc = bacc.Bacc("TRN2", target_bir_lowering=False, debug=False,
                   num_devices=N_CORES)

    xT = nc.dram_tensor("xT", [EMB, N], F16, kind="ExternalInput").ap()
    wqT = nc.dram_tensor("wqT", [EMB, DQ], F16, kind="ExternalInput").ap()
    wkT = nc.dram_tensor("wkT", [EMB, DQ], F16, kind="ExternalInput").ap()
    wvT = nc.dram_tensor("wvT", [EMB, DQ], F16, kind="ExternalInput").ap()
    woT = nc.dram_tensor("woT", [DQ, EMB], F16, kind="ExternalInput").ap()
    bqd = nc.dram_tensor("bq_s", [DQ], F32, kind="ExternalInput").ap()
    bkd = nc.dram_tensor("bk_s", [DQ], F32, kind="ExternalInput").ap()
    # fp16 partials: host sums 4 of them in fp32; quantization of the
    # partial (|.| ~ 1, ulp ~ 1e-3) adds ~1e-3 absmax-relative error --
    # well under the 2e-2 gate -- and halves the store traffic + tail.
    out_part = nc.dram_tensor("out_part", [N, EMB], F16,
                              kind="ExternalOutput").ap()

    with tile.TileContext(nc) as tc:
        with (
            tc.tile_pool(name="const", bufs=1) as const,
            tc.tile_pool(name="persist", bufs=1) as persist,
            tc.tile_pool(name="epool", bufs=3) as epool,
            tc.tile_pool(name="npool", bufs=2) as npool,
            tc.tile_pool(name="opool", bufs=4) as opool,
            tc.tile_pool(name="o0pool", bufs=1) as o0pool,
            # PSUM static budget (8 banks): pp 2 + s0 2 + s1 2 + c0 1 + c1 1
            tc.tile_pool(name="ppool", bufs=2, space="PSUM") as ppool,
            tc.tile_pool(name="spool", bufs=1, space="PSUM") as spool,
            tc.tile_pool(name="cpool", bufs=1, space="PSUM") as cpool,
        ):
            # ---- constants + resident x ----
            # first projection group needs wk chunks + x n-slice 0 only; those
            # DMAs go first, fine-grained, spread across the Sync / GpSimd /
            # Scalar queues so no single sequencer's DGE time serializes the
            # startup. x n-slice 0 is 8 small chunk DMAs (fast first arrival);
            # slices 1-3 ride 8 bigger strided slab DMAs.
            wq_sb = const.tile([128, KC, DQ], F16, tag="wq")
            wk_sb = const.tile([128, KC, DQ], F16, tag="wk")
            wv_sb = const.tile([128, KC, DQ], F16, tag="wv")
            x_sb = const.tile([128, KC, N], F16, tag="x")
            xTr = xT.rearrange("(k p) n -> k p n", p=128)
            # x chunks arrive in exactly the order the projection prefix
            # consumes them (n-major), spread over all three DMA-capable
            # queues (Sync/GpSimd/Scalar) so no single sequencer's ~0.6us
            # per-DMA DGE time serializes the startup: n-slice 0 is split
            # even/odd-k across GpSimd/Scalar for the fastest first arrival.
            bk_sb = const.tile([128, 2], F32, tag="bk")
            bq_sb = const.tile([128, 2], F32, tag="bq")
            nc.scalar.dma_start(out=bk_sb, in_=bkd.rearrange("(m p) -> p m", p=128))
            nc.scalar.dma_start(out=bq_sb, in_=bqd.rearrange("(m p) -> p m", p=128))
            for k in range(KC):
                nc.sync.dma_start(out=wk_sb[:, k, :], in_=wkT.rearrange(
                    "(k p) d -> k p d", p=128)[k])
                nc.gpsimd.dma_start(out=x_sb[:, k, 0:NQ], in_=xTr[k, :, 0:NQ])
                nc.scalar.dma_start(out=wv_sb[:, k, :], in_=wvT.rearrange(
                    "(k p) d -> k p d", p=128)[k])
            for k in range(KC):
                nc.sync.dma_start(out=wq_sb[:, k, :], in_=wqT.rearrange(
                    "(k p) d -> k p d", p=128)[k])
                nc.gpsimd.dma_start(out=x_sb[:, k, 2 * NQ:3 * NQ],
                                    in_=xTr[k, :, 2 * NQ:3 * NQ])
                nc.scalar.dma_start(out=x_sb[:, k, NQ:2 * NQ],
                                    in_=xTr[k, :, NQ:2 * NQ])
            for k in range(KC):
                nc.sync.dma_start(out=x_sb[:, k, 3 * NQ:N],
                                  in_=xTr[k, :, 3 * NQ:N])
            # wo is needed only by the out-projection (~100us in) — deferred
            # into the filler stream to keep startup queues clear
            wo_sb = const.tile([128, 2, EMB], F16, tag="wo")

            # ---- persistent activations ----
            qT = [persist.tile([128, N], F16, tag=f"qT{p}", name=f"qT{p}") for p in range(2)]
            kT = [persist.tile([128, N], F16, tag=f"kT{p}", name=f"kT{p}") for p in range(2)]
            ctxT = [persist.tile([128, N], F16, tag=f"ctxT{p}", name=f"ctxT{p}") for p in range(2)]
            # V for all 4 local heads: [nk-part, t, head*65 + (0:64 | ones)]
            v_all = persist.tile([128, NT, 4 * (HD + 1)], F16, tag="v")
            for h in range(4):
                nc.vector.memset(v_all[:, :, h * 65 + 64], 1.0)

            add, mult = mybir.AluOpType.add, mybir.AluOpType.mult

            # ---- projection building blocks ----
            # Each is one PSUM accumulation group on the double-buffered pp
            # tag, small enough to slot between attention groups.
            def kq_group(p, n, wsb, bsb, dst):
                ps = ppool.tile([128, NQ], F32, tag="pp", name="kqp")
                for k in range(KC):
                    nc.tensor.matmul(
                        ps, wsb[:, k, p * 128:(p + 1) * 128],
                        x_sb[:, k, n * NQ:(n + 1) * NQ],
                        start=(k == 0), stop=(k == KC - 1))
                nc.vector.tensor_tensor(
                    out=dst[p][:, n * NQ:(n + 1) * NQ], in0=ps,
                    in1=bsb[:, p:p + 1].broadcast_to([128, NQ]), op=add)

            def v_group(n, tl):
                # V for ALL 4 local heads at once (256-col moving wv): done
                # in the prefix so pair-1's share never burdens the p0
                # attention windows
                t = n * 4 + tl
                ps = ppool.tile([128, DQ], F32, tag="pp", name="vp")
                for k in range(KC):
                    nc.tensor.matmul(
                        ps, x_sb[:, k, t * 128:(t + 1) * 128],
                        wv_sb[:, k, :],
                        start=(k == 0), stop=(k == KC - 1))
                vv = v_all[:, t, :].rearrange("p (h c) -> p h c", c=65)
                nc.vector.tensor_copy(
                    out=vv[:, :, 0:64],
                    in_=ps.rearrange("p (h c) -> p h c", c=64))

            # Minimal cold prefix: the PE runs at ~half clock until ~3us of
            # continuous execution, so only the work attention j=0 strictly
            # needs before its first items runs up front — K(n=0), Q(n=0),
            # V(t=0,1). Everything else streams through the hot attention
            # windows as fillers (emission order constraints handled by the
            # per-window filler lists below).
            kq_group(0, 0, wk_sb, bk_sb, kT)
            kq_group(0, 0, wq_sb, bq_sb, qT)
            v_group(0, 0)
            v_group(0, 1)

            # ---- out-projection ----
            # one (m, eo) parcel: both head-pair passes accumulate into a
            # single pp-tag PSUM group, then one copy to SBUF + store.
            def out_proj_parcel(m, eo):
                po = ppool.tile([128, NQ], F32, tag="pp", name="po")
                for kp in range(2):
                    nc.tensor.matmul(
                        po, ctxT[kp][:, m * 128:(m + 1) * 128],
                        wo_sb[:, kp, eo * NQ:(eo + 1) * NQ],
                        start=(kp == 0), stop=(kp == 1))
                o = opool.tile([128, NQ], F16, tag="o", name="o")
                nc.vector.tensor_copy(o, po)
                nc.gpsimd.dma_start(
                    out=out_part[m * 128:(m + 1) * 128, eo * NQ:(eo + 1) * NQ],
                    in_=o)

            # The LAST window's m-chunks split the parcel: the ctxT0 half
            # runs during the window (ctxT0 is final since p0), leaving only
            # one matmul + add + store per parcel on the serial tail.
            o0s = {}

            def oproj_kp0(m, eo):
                po = ppool.tile([128, NQ], F32, tag="pp", name="po")
                nc.tensor.matmul(
                    po, ctxT[0][:, m * 128:(m + 1) * 128],
                    wo_sb[:, 0, eo * NQ:(eo + 1) * NQ], start=True, stop=True)
                o0 = o0pool.tile([128, NQ], F16, tag=f"o0_{m}_{eo}", name="o0")
                nc.vector.tensor_copy(o0, po)
                o0s[(m, eo)] = o0

            def oproj_kp1(m, eo):
                po = ppool.tile([128, NQ], F32, tag="pp", name="po")
                nc.tensor.matmul(
                    po, ctxT[1][:, m * 128:(m + 1) * 128],
                    wo_sb[:, 1, eo * NQ:(eo + 1) * NQ], start=True, stop=True)
                o = opool.tile([128, NQ], F16, tag="o", name="o")
                nc.vector.tensor_tensor(out=o, in0=o0s[(m, eo)], in1=po,
                                        op=add)
                nc.gpsimd.dma_start(
                    out=out_part[m * 128:(m + 1) * 128, eo * NQ:(eo + 1) * NQ],
                    in_=o)

            # ---- attention (per head-pair p, nq window of 512/256) ----
            # Software-pipelined: ctx matmuls for work item u are emitted
            # after the S/exp of item u+1, so PE always has ready work while
            # ACT streams 1024-wide exps; heads alternate as the natural PSUM
            # ping-pong for the S tiles. Projection/out-proj parcels are
            # popped between work items from per-window filler lists whose
            # ordering respects emission deadlines: K(n)/V(t)/Q(j) parcels
            # are always emitted before the first S/ctx that reads their
            # output. The final p=1 window is split into two 256-wide halves
            # so the serial tail covers half as many m-chunks.
            from collections import deque
            fillers = deque()

            def K0(n):
                return lambda: kq_group(0, n, wk_sb, bk_sb, kT)

            def Q0(n):
                return lambda: kq_group(0, n, wq_sb, bq_sb, qT)

            def K1(n):
                return lambda: kq_group(1, n, wk_sb, bk_sb, kT)

            def Q1(n):
                return lambda: kq_group(1, n, wq_sb, bq_sb, qT)

            def V(n, tl):
                return lambda: v_group(n, tl)

            wo_dma = lambda: nc.sync.dma_start(  # noqa: E731
                out=wo_sb, in_=woT.rearrange("(k p) e -> p k e", p=128))

            # per-window static filler lists (window key = (p, index))
            sched = {
                (0, 0): ([V(0, 2), V(0, 3), K0(1)]
                         + [V(1, tl) for tl in range(4)] + [K0(2)]
                         + [V(2, tl) for tl in range(4)] + [K0(3)]
                         + [V(3, tl) for tl in range(2)] + [Q0(1)]
                         + [V(3, tl) for tl in range(2, 4)]),
                (0, 1): [Q0(2), K1(0), Q1(0)],
                (0, 2): [Q0(3), K1(1), wo_dma],
                (0, 3): [K1(2)],
                (1, 0): [K1(3), Q1(1)],
                (1, 1): [Q1(2)],
                (1, 2): [Q1(3)],
                (1, 3): [lambda m=m, eo=eo: oproj_kp0(m, eo)
                         for m in range(14, 16) for eo in range(2)],
            }

            for p in range(2):
                if p == 0:
                    wins = [(jq * NQA, NQA) for jq in range(NJA)]
                else:
                    # final window split in two 256-halves: the serial tail
                    # (normalize chain + kp1 finishers + stores) covers only
                    # 2 m-chunks
                    wins = [(jq * NQA, NQA) for jq in range(NJA - 1)]
                    wins += [(N - NQA, NQA // 2), (N - NQA // 2, NQA // 2)]
                for jw, (q0, w) in enumerate(wins):
                    # projection fillers go to the FRONT so their DVE tails
                    # retire well before the next window's S reads them;
                    # out-proj parcels (appended at the previous window's
                    # end) drain after them.
                    statics = sched.get((p, jw), [])
                    fillers.extendleft(reversed(statics))
                    n_static = len(statics)
                    cps = [cpool.tile([HD + 1, w], F32, tag=f"c{h}",
                                      name=f"c{h}") for h in range(2)]

                    def s_mms(g, h, q0=q0, w=w):
                        lo = 64 * h
                        sp = spool.tile([128, GK, w], F32,
                                        tag=f"s{h}", name=f"s{h}")
                        for i, t in enumerate(g):
                            nc.tensor.matmul(
                                sp[:, i, :],
                                kT[p][lo:lo + 64, t * 128:(t + 1) * 128],
                                qT[p][lo:lo + 64, q0:q0 + w],
                                start=True, stop=True)
                        return sp

                    def exp_act(sp, g, h, w=w):
                        e = epool.tile([128, GK, w], F16,
                                       tag=f"e{h}", name=f"e{h}")
                        nc.scalar.activation(e, sp, FP.Exp, scale=SCALE)
                        return e

                    def ctx_mms(e, g, h):
                        hloc = 2 * p + h
                        for i, t in enumerate(g):
                            nc.tensor.matmul(
                                cps[h],
                                v_all[:, t, hloc * 65:(hloc + 1) * 65],
                                e[:, i, :],
                                start=(t == 0), stop=(t == NT - 1))

                    work = []
                    for gi in range(NG):
                        for h in range(2):
                            work.append((tuple(range(gi * GK, (gi + 1) * GK)), h))
                    n_pop = len(fillers)  # drain this window's list fully
                    popped = 0
                    prev = None
                    for wi, (g, h) in enumerate(work):
                        # order within an item: S(u) | filler | ctx(u-1) —
                        # the filler between S and ctx gives exp(u-1) the
                        # extra headroom to finish before the in-order PE
                        # queue reaches ctx(u-1), and keeps S(u+1) far enough
                        # behind exp(u-1)'s release of the S-psum tag.
                        sp = s_mms(g, h)
                        cur = (exp_act(sp, g, h), g, h)
                        # static projection fillers drain eagerly (one per
                        # item) so their DVE tails retire early; parcels are
                        # paced to finish ~4 items before the window's end so
                        # their PSUM->SBUF copies don't collide with the
                        # normalize chain on DVE
                        target = max(min(wi + 1, n_static),
                                     (wi + 1) * n_pop // max(1, len(work) - 4))
                        while fillers and popped < min(n_pop, target):
                            fillers.popleft()()
                            popped += 1
                        if prev is not None:
                            ctx_mms(*prev)
                        prev = cur
                    ctx_mms(*prev)
                    while fillers and popped < n_pop:
                        fillers.popleft()()
                        popped += 1

                    # normalize: ctx^T[0:64] * (1 / rowsum); rowsum in row 64.
                    # Two quick DVE copies (rowsum row -> partition 0 for
                    # partition_broadcast, ctx body -> SBUF) release the ctx
                    # PSUM bank ~1.5us after the last ctx matmul so the next
                    # window's ctx accumulation never stalls on it; the
                    # broadcast + reciprocal then run off the critical path.
                    last = (p == 1 and jw == len(wins) - 1)
                    css = [None, None]
                    for h in range(2):
                        rs = npool.tile([1, w], F32, tag="rs", name="rs")
                        nc.vector.tensor_copy(rs, cps[h][64:65, :])
                        if not last:
                            # mid-kernel: stage ctx to SBUF to release the c
                            # PSUM bank before the next window needs it
                            cs = npool.tile([64, w], F32, tag="cs", name="cs")
                            nc.vector.tensor_copy(cs, cps[h][0:64, :])
                            css[h] = cs
                        rb = npool.tile([64, w], F32, tag="rb", name="rb")
                        nc.gpsimd.partition_broadcast(rb, rs)
                        rc = npool.tile([64, w], F32, tag="rc", name="rc")
                        nc.vector.reciprocal_approx_fast(out=rc, in_=rb)
                        # last window: multiply straight out of PSUM — the
                        # shorter chain gates the tail's out-proj finishers
                        nc.vector.tensor_tensor(
                            out=ctxT[p][h * 64:(h + 1) * 64, q0:q0 + w],
                            in0=css[h] if not last else cps[h][0:64, :],
                            in1=rc, op=mult)
                    if p == 1:
                        # ctxT1 columns for this window are final -> out-proj
                        # parcels for the covered m-chunks can run (the last
                        # window's parcels are the short kp1 finishers)
                        for m in range(q0 // 128, (q0 + w) // 128):
                            for eo in range(2):
                                f = (out_proj_parcel
                                     if jw < len(wins) - 1 else oproj_kp1)
                                fillers.append(lambda m=m, eo=eo, f=f: f(m, eo))
            while fillers:
                fillers.popleft()()

    nc.compile()
    return nc


_NC_CACHE = {}


def _get_program():
    if "nc" not in _NC_CACHE:
        _NC_CACHE["nc"] = build_program()
    return _NC_CACHE["nc"]


def make_in_maps(x, Wq, bq, Wk, bk, Wv, bv, Wo, bo):
    x = np.asarray(x)
    xTs = [np.ascontiguousarray(x[b].T.astype(np.float16)) for b in range(B)]
    in_maps = []
    for c in range(N_CORES):
        b, hg = divmod(c, TP)
        sl = slice(hg * DQ, (hg + 1) * DQ)
        in_maps.append({
            "xT": xTs[b],
            "wqT": np.ascontiguousarray(np.asarray(Wq, np.float16)[sl, :].T),
            "wkT": np.ascontiguousarray(np.asarray(Wk, np.float16)[sl, :].T),
            "wvT": np.ascontiguousarray(np.asarray(Wv, np.float16)[sl, :].T),
            "woT": np.ascontiguousarray(np.asarray(Wo, np.float16)[:, sl].T),
            "bq_s": np.ascontiguousarray(np.asarray(bq, np.float32)[sl]),
            "bk_s": np.ascontiguousarray(np.asarray(bk, np.float32)[sl]),
        })
    return in_maps


def assemble_output(results, Wv_bias_term):
    out = np.empty((B, N, EMB), np.float32)
    for b in range(B):
        acc = results[b * TP]["out_part"].astype(np.float32)
        for g in range(1, TP):
            acc = acc + results[b * TP + g]["out_part"]
        out[b] = acc + Wv_bias_term
    return out


def kernel(x, Wq, bq, Wk, bk, Wv, bv, Wo, bo):
    nc = _get_program()
    in_maps = make_in_maps(x, Wq, bq, Wk, bk, Wv, bv, Wo, bo)
    res = run_bass_kernel_spmd(nc, in_maps, list(range(N_CORES)))
    bias_term = (np.asarray(bv, np.float32) @ np.asarray(Wo, np.float32).T
                 + np.asarray(bo, np.float32))
    return assemble_output(res.results, bias_term)


# revision 44
# speedup vs baseline: 1.0269x; 1.0269x over previous
"""Multi-head self-attention (b=2, n=2048, emb=1024, heads=16) on 8 trn2 cores.

Sharding: core c = (b, hg) with b = c // 4, hg = c % 4. Data parallel over
batch, tensor parallel over head-groups (4 heads / 256 emb-cols per core).
Each core computes Q/K/V projections for its heads, full attention for its
heads, and a partial output projection ctx_hg @ Wo[:, hg_slice].T of shape
[2048, 1024]. The host sums the 4 partials per batch (Megatron row-parallel
reduce done on host) and adds the rank-1 bias term bv @ Wo.T + bo.

Device layout notes:
- x^T [emb, n] fp16 (4MB = 32KB/partition) is loaded ONCE and stays resident
  in SBUF; both head-pairs' projections read it, so there is no mid-kernel
  x DMA traffic and no DMA wait can head-of-line-block the in-order PE queue.
- DMA issue is split across the Sync and GpSimd queues so sequencer DGE time
  doesn't serialize the startup (weights on Sync, x + output stores on
  GpSimd).
- Q^T, K^T are produced in [dq, n] layout (dq = head-major), V in natural
  [n, dv] layout augmented with a ones column per head -> the ctx matmul
  ctxT[65, nq] = V_aug^T @ E^T produces softmax row-sums in row 64 for free.
- Attention runs on 512-wide nq chunks: S matmuls, 1024-wide exps straight
  out of PSUM, and ctx matmuls all stream 512 columns, halving the PE
  instruction count (and its stall overhead) vs 256-wide chunks.
- Softmax normalization: row-sum staged to partition 0 (DVE), broadcast
  (GpSimd), reciprocal_approx_fast (single custom-DVE op, ~5x faster than
  InstReciprocal; denominators are strictly positive sums of exps so the
  approx's edge cases can't occur), then one fused multiply straight out of
  the ctx PSUM bank into ctxT fp16.
- Output projection accumulates both head-pair contributions into one PSUM
  group (2 matmuls back-to-back), needing a single PSUM->SBUF copy and no
  SBUF accumulator array.
- All matmuls run in float16 (1 cyc/col on PE; 10-bit mantissa keeps the
  overall error ~7e-4 scale-relative, validated vs fp32).
- q/k biases are added on-device (fused into the PSUM->SBUF copy);
  v/o biases are exactly the rank-1 host-side term above.
"""

import os
import sys

for _p in ("/opt/trn_rl_repo", "/root/.axon_site/_ro/trn_rl_repo"):
    if os.path.isdir(_p) and _p not in sys.path:
        sys.path.append(_p)

import numpy as np

import concourse.bass as bass  # noqa: F401  (engine types pulled via nc)
import concourse.mybir as mybir
import concourse.tile as tile
from concourse import bacc
from concourse.bass_utils import run_bass_kernel_spmd

B, N, EMB, HEADS, HD = 2, 2048, 1024, 16, 64
N_CORES = 8
TP = 4                      # head-group shards per batch
DQ = EMB // TP              # 256 emb-cols (4 heads) per core
SCALE = HD ** -0.5          # 0.125

F32 = mybir.dt.float32
F16 = mybir.dt.float16
FP = mybir.ActivationFunctionType

NQ = 512                    # nq chunk for projections / out-proj
NJ = N // NQ                # 4 nq chunks
NQA = 512                   # nq chunk for attention
NJA = N // NQA              # 4 attention nq chunks
NKC = 128                   # nk chunk (ctx contraction)
NT = N // NKC               # 16 nk chunks
KC = EMB // 128             # 8 e chunks
GK = 2                      # nk chunks per S-psum group (1024-wide exps)
NG = NT // GK               # 8 groups per (head, j)


def build_program():
    """Build + compile the single SPMD program all 8 cores run."""
    nc = bacc.Bacc("TRN2", target_bir_lowering=False, debug=False,
                   num_devices=N_CORES)

    xT = nc.dram_tensor("xT", [EMB, N], F16, kind="ExternalInput").ap()
    wqT = nc.dram_tensor("wqT", [EMB, DQ], F16, kind="ExternalInput").ap()
    wkT = nc.dram_tensor("wkT", [EMB, DQ], F16, kind="ExternalInput").ap()
    wvT = nc.dram_tensor("wvT", [EMB, DQ], F16, kind="ExternalInput").ap()
    woT = nc.dram_tensor("woT", [DQ, EMB], F16, kind="ExternalInput").ap()
    bqd = nc.dram_tensor("bq_s", [DQ], F32, kind="ExternalInput").ap()
    bkd = nc.dram_tensor("bk_s", [DQ], F32, kind="ExternalInput").ap()
    # fp16 partials: host sums 4 of them in fp32; quantization of the
    # partial (|.| ~ 1, ulp ~ 1e-3) adds ~1e-3 absmax-relative error --
    # well under the 2e-2 gate -- and halves the store traffic + tail.
    out_part = nc.dram_tensor("out_part", [N, EMB], F16,
                              kind="ExternalOutput").ap()

    with tile.TileContext(nc) as tc:
        with (
            tc.tile_pool(name="const", bufs=1) as const,
            tc.tile_pool(name="persist", bufs=1) as persist,
            tc.tile_pool(name="epool", bufs=3) as epool,
            tc.tile_pool(name="npool", bufs=2) as npool,
            tc.tile_pool(name="opool", bufs=4) as opool,
            # PSUM static budget (8 banks): pp 2 + s0 2 + s1 2 + c0 1 + c1 1
            tc.tile_pool(name="ppool", bufs=2, space="PSUM") as ppool,
            tc.tile_pool(name="spool", bufs=1, space="PSUM") as spool,
            tc.tile_pool(name="cpool", bufs=1, space="PSUM") as cpool,
        ):
            # ---- constants + resident x ----
            # first projection group needs wk chunks + x n-slice 0 only; those
            # DMAs go first, fine-grained, spread across the Sync / GpSimd /
            # Scalar queues so no single sequencer's DGE time serializes the
            # startup. x n-slice 0 is 8 small chunk DMAs (fast first arrival);
            # slices 1-3 ride 8 bigger strided slab DMAs.
            wq_sb = const.tile([128, KC, DQ], F16, tag="wq")
            wk_sb = const.tile([128, KC, DQ], F16, tag="wk")
            wv_sb = const.tile([128, KC, DQ], F16, tag="wv")
            x_sb = const.tile([128, KC, N], F16, tag="x")
            xTr = xT.rearrange("(k p) n -> k p n", p=128)
            # x chunks arrive in exactly the order the projection prefix
            # consumes them (n-major), spread over all three DMA-capable
            # queues (Sync/GpSimd/Scalar) so no single sequencer's ~0.6us
            # per-DMA DGE time serializes the startup: n-slice 0 is split
            # even/odd-k across GpSimd/Scalar for the fastest first arrival.
            bk_sb = const.tile([128, 2], F32, tag="bk")
            bq_sb = const.tile([128, 2], F32, tag="bq")
            # x n-slice 0 goes FIRST, split even/odd-k across GpSimd/Scalar:
            # the whole slice lands before K(0)'s matmuls reach it, so the
            # cold-clock prefix is never DMA-issue-paced. Biases + wq ride
            # Sync behind wk; wv follows the n0 chunks on Scalar.
            for k in range(0, KC, 2):
                nc.gpsimd.dma_start(out=x_sb[:, k, 0:NQ], in_=xTr[k, :, 0:NQ])
                nc.scalar.dma_start(out=x_sb[:, k + 1, 0:NQ],
                                    in_=xTr[k + 1, :, 0:NQ])
            for k in range(KC):
                nc.sync.dma_start(out=wk_sb[:, k, :], in_=wkT.rearrange(
                    "(k p) d -> k p d", p=128)[k])
                nc.scalar.dma_start(out=wv_sb[:, k, :], in_=wvT.rearrange(
                    "(k p) d -> k p d", p=128)[k])
            nc.sync.dma_start(out=bk_sb, in_=bkd.rearrange("(m p) -> p m", p=128))
            nc.sync.dma_start(out=bq_sb, in_=bqd.rearrange("(m p) -> p m", p=128))
            for k in range(KC):
                nc.sync.dma_start(out=wq_sb[:, k, :], in_=wqT.rearrange(
                    "(k p) d -> k p d", p=128)[k])
                nc.gpsimd.dma_start(out=x_sb[:, k, 2 * NQ:3 * NQ],
                                    in_=xTr[k, :, 2 * NQ:3 * NQ])
                nc.scalar.dma_start(out=x_sb[:, k, NQ:2 * NQ],
                                    in_=xTr[k, :, NQ:2 * NQ])
            for k in range(KC):
                nc.sync.dma_start(out=x_sb[:, k, 3 * NQ:N],
                                  in_=xTr[k, :, 3 * NQ:N])
            # wo is needed only by the out-projection (~100us in) — deferred
            # into the filler stream to keep startup queues clear
            wo_sb = const.tile([128, 2, EMB], F16, tag="wo")

            # ---- persistent activations ----
            qT = [persist.tile([128, N], F16, tag=f"qT{p}", name=f"qT{p}") for p in range(2)]
            kT = [persist.tile([128, N], F16, tag=f"kT{p}", name=f"kT{p}") for p in range(2)]
            ctxT = [persist.tile([128, N], F16, tag=f"ctxT{p}", name=f"ctxT{p}") for p in range(2)]
            # V for all 4 local heads: [nk-part, t, head*65 + (0:64 | ones)]
            v_all = persist.tile([128, NT, 4 * (HD + 1)], F16, tag="v")
            for h in range(4):
                nc.vector.memset(v_all[:, :, h * 65 + 64], 1.0)

            add, mult = mybir.AluOpType.add, mybir.AluOpType.mult

            # ---- projection building blocks ----
            # Each is one PSUM accumulation group on the double-buffered pp
            # tag, small enough to slot between attention groups.
            def kq_group(p, n, wsb, bsb, dst):
                ps = ppool.tile([128, NQ], F32, tag="pp", name="kqp")
                for k in range(KC):
                    nc.tensor.matmul(
                        ps, wsb[:, k, p * 128:(p + 1) * 128],
                        x_sb[:, k, n * NQ:(n + 1) * NQ],
                        start=(k == 0), stop=(k == KC - 1))
                nc.vector.tensor_tensor(
                    out=dst[p][:, n * NQ:(n + 1) * NQ], in0=ps,
                    in1=bsb[:, p:p + 1].broadcast_to([128, NQ]), op=add)

            def v_group(n, tl):
                # V for ALL 4 local heads at once (256-col moving wv): done
                # in the prefix so pair-1's share never burdens the p0
                # attention windows
                t = n * 4 + tl
                ps = ppool.tile([128, DQ], F32, tag="pp", name="vp")
                for k in range(KC):
                    nc.tensor.matmul(
                        ps, x_sb[:, k, t * 128:(t + 1) * 128],
                        wv_sb[:, k, :],
                        start=(k == 0), stop=(k == KC - 1))
                vv = v_all[:, t, :].rearrange("p (h c) -> p h c", c=65)
                nc.vector.tensor_copy(
                    out=vv[:, :, 0:64],
                    in_=ps.rearrange("p (h c) -> p h c", c=64))

            # Minimal cold prefix: the PE runs at ~half clock until ~3us of
            # continuous execution, so only the work attention j=0 strictly
            # needs before its first items runs up front — K(n=0), Q(n=0),
            # V(t=0,1). Everything else streams through the hot attention
            # windows as fillers (emission order constraints handled by the
            # per-window filler lists below).
            kq_group(0, 0, wk_sb, bk_sb, kT)
            kq_group(0, 0, wq_sb, bq_sb, qT)
            v_group(0, 0)
            v_group(0, 1)

            # ---- out-projection ----
            # one (m, eo) parcel: both head-pair passes accumulate into a
            # single pp-tag PSUM group, then one copy to SBUF + store.
            def out_proj_parcel(m, eo):
                po = ppool.tile([128, NQ], F32, tag="pp", name="po")
                for kp in range(2):
                    nc.tensor.matmul(
                        po, ctxT[kp][:, m * 128:(m + 1) * 128],
                        wo_sb[:, kp, eo * NQ:(eo + 1) * NQ],
                        start=(kp == 0), stop=(kp == 1))
                o = opool.tile([128, NQ], F16, tag="o", name="o")
                nc.vector.tensor_copy(o, po)
                nc.gpsimd.dma_start(
                    out=out_part[m * 128:(m + 1) * 128, eo * NQ:(eo + 1) * NQ],
                    in_=o)

            # The LAST window's m-chunks split the parcel: the ctxT0 half
            # runs during the window (ctxT0 is final since p0), leaving only
            # one matmul + add + store per parcel on the serial tail.
            o0s = {}

            def oproj_kp0(m, eo):
                po = ppool.tile([128, NQ], F32, tag="pp", name="po")
                nc.tensor.matmul(
                    po, ctxT[0][:, m * 128:(m + 1) * 128],
                    wo_sb[:, 0, eo * NQ:(eo + 1) * NQ], start=True, stop=True)
                o0 = opool.tile([128, NQ], F32, tag=f"o0_{m}_{eo}", name="o0")
                nc.vector.tensor_copy(o0, po)
                o0s[(m, eo)] = o0

            def oproj_kp1(m, eo):
                po = ppool.tile([128, NQ], F32, tag="pp", name="po")
                nc.tensor.matmul(
                    po, ctxT[1][:, m * 128:(m + 1) * 128],
                    wo_sb[:, 1, eo * NQ:(eo + 1) * NQ], start=True, stop=True)
                o = opool.tile([128, NQ], F16, tag="o", name="o")
                nc.vector.tensor_tensor(out=o, in0=o0s[(m, eo)], in1=po,
                                        op=add)
                nc.gpsimd.dma_start(
                    out=out_part[m * 128:(m + 1) * 128, eo * NQ:(eo + 1) * NQ],
                    in_=o)

            # ---- attention (per head-pair p, nq window of 512/256) ----
            # Software-pipelined: ctx matmuls for work item u are emitted
            # after the S/exp of item u+1, so PE always has ready work while
            # ACT streams 1024-wide exps; heads alternate as the natural PSUM
            # ping-pong for the S tiles. Projection/out-proj parcels are
            # popped between work items from per-window filler lists whose
            # ordering respects emission deadlines: K(n)/V(t)/Q(j) parcels
            # are always emitted before the first S/ctx that reads their
            # output. The final p=1 window is split into two 256-wide halves
            # so the serial tail covers half as many m-chunks.
            from collections import deque
            fillers = deque()

            def K0(n):
                return lambda: kq_group(0, n, wk_sb, bk_sb, kT)

            def Q0(n):
                return lambda: kq_group(0, n, wq_sb, bq_sb, qT)

            def K1(n):
                return lambda: kq_group(1, n, wk_sb, bk_sb, kT)

            def Q1(n):
                return lambda: kq_group(1, n, wq_sb, bq_sb, qT)

            def V(n, tl):
                return lambda: v_group(n, tl)

            wo_dma = lambda: nc.sync.dma_start(  # noqa: E731
                out=wo_sb, in_=woT.rearrange("(k p) e -> p k e", p=128))

            # per-window static filler lists (window key = (p, index))
            sched = {
                (0, 0): ([V(0, 2), V(0, 3), K0(1)]
                         + [V(1, tl) for tl in range(4)] + [K0(2)]
                         + [V(2, tl) for tl in range(4)] + [K0(3)]
                         + [V(3, tl) for tl in range(2)] + [Q0(1)]
                         + [V(3, tl) for tl in range(2, 4)]),
                (0, 1): [Q0(2), K1(0), Q1(0)],
                (0, 2): [Q0(3), K1(1), wo_dma],
                (0, 3): [K1(2)],
                (1, 0): [K1(3), Q1(1)],
                (1, 1): [Q1(2)],
                (1, 2): [Q1(3)],
                (1, 3): [lambda m=m, eo=eo: oproj_kp0(m, eo)
                         for m in range(14, 16) for eo in range(2)],
            }

            for p in range(2):
                if p == 0:
                    wins = [(jq * NQA, NQA) for jq in range(NJA)]
                else:
                    # final window split in two 256-halves: the serial tail
                    # (normalize chain + kp1 finishers + stores) covers only
                    # 2 m-chunks
                    wins = [(jq * NQA, NQA) for jq in range(NJA - 1)]
                    wins += [(N - NQA, NQA // 2), (N - NQA // 2, NQA // 2)]
                for jw, (q0, w) in enumerate(wins):
                    # projection fillers go to the FRONT so their DVE tails
                    # retire well before the next window's S reads them;
                    # out-proj parcels (appended at the previous window's
                    # end) drain after them.
                    statics = sched.get((p, jw), [])
                    fillers.extendleft(reversed(statics))
                    n_static = len(statics)
                    cps = [cpool.tile([HD + 1, w], F32, tag=f"c{h}",
                                      name=f"c{h}") for h in range(2)]

                    def s_mms(g, h, q0=q0, w=w):
                        lo = 64 * h
                        sp = spool.tile([128, GK, w], F32,
                                        tag=f"s{h}", name=f"s{h}")
                        for i, t in enumerate(g):
                            nc.tensor.matmul(
                                sp[:, i, :],
                                kT[p][lo:lo + 64, t * 128:(t + 1) * 128],
                                qT[p][lo:lo + 64, q0:q0 + w],
                                start=True, stop=True)
                        return sp

                    def exp_act(sp, g, h, w=w):
                        e = epool.tile([128, GK, w], F16,
                                       tag=f"e{h}", name=f"e{h}")
                        nc.scalar.activation(e, sp, FP.Exp, scale=SCALE)
                        return e

                    def ctx_mms(e, g, h):
                        hloc = 2 * p + h
                        for i, t in enumerate(g):
                            nc.tensor.matmul(
                                cps[h],
                                v_all[:, t, hloc * 65:(hloc + 1) * 65],
                                e[:, i, :],
                                start=(t == 0), stop=(t == NT - 1))

                    work = []
                    for gi in range(NG):
                        for h in range(2):
                            work.append((tuple(range(gi * GK, (gi + 1) * GK)), h))
                    n_pop = len(fillers)  # drain this window's list fully
                    popped = 0
                    prev = None
                    for wi, (g, h) in enumerate(work):
                        # order within an item: S(u) | filler | ctx(u-1) —
                        # the filler between S and ctx gives exp(u-1) the
                        # extra headroom to finish before the in-order PE
                        # queue reaches ctx(u-1), and keeps S(u+1) far enough
                        # behind exp(u-1)'s release of the S-psum tag.
                        sp = s_mms(g, h)
                        cur = (exp_act(sp, g, h), g, h)
                        # static projection fillers drain eagerly (one per
                        # item) so their DVE tails retire early; parcels are
                        # paced to finish ~4 items before the window's end so
                        # their PSUM->SBUF copies don't collide with the
                        # normalize chain on DVE
                        target = max(min(wi + 1, n_static),
                                     (wi + 1) * n_pop // max(1, len(work) - 4))
                        while fillers and popped < min(n_pop, target):
                            fillers.popleft()()
                            popped += 1
                        if prev is not None:
                            ctx_mms(*prev)
                        prev = cur
                    ctx_mms(*prev)
                    while fillers and popped < n_pop:
                        fillers.popleft()()
                        popped += 1

                    # normalize: ctx^T[0:64] * (1 / rowsum); rowsum in row 64.
                    # Two quick DVE copies (rowsum row -> partition 0 for
                    # partition_broadcast, ctx body -> SBUF) release the ctx
                    # PSUM bank ~1.5us after the last ctx matmul so the next
                    # window's ctx accumulation never stalls on it; the
                    # broadcast + reciprocal then run off the critical path.
                    last = (p == 1 and jw == len(wins) - 1)
                    css = [None, None]
                    for h in range(2):
                        rs = npool.tile([1, w], F32, tag="rs", name="rs")
                        nc.vector.tensor_copy(rs, cps[h][64:65, :])
                        if not last:
                            # mid-kernel: stage ctx to SBUF to release the c
                            # PSUM bank before the next window needs it
                            cs = npool.tile([64, w], F32, tag="cs", name="cs")
                            nc.vector.tensor_copy(cs, cps[h][0:64, :])
                            css[h] = cs
                        rb = npool.tile([64, w], F32, tag="rb", name="rb")
                        nc.gpsimd.partition_broadcast(rb, rs)
                        rc = npool.tile([64, w], F32, tag="rc", name="rc")
                        nc.vector.reciprocal_approx_fast(out=rc, in_=rb)
                        # last window: multiply straight out of PSUM — the
                        # shorter chain gates the tail's out-proj finishers
                        nc.vector.tensor_tensor(
                            out=ctxT[p][h * 64:(h + 1) * 64, q0:q0 + w],
                            in0=css[h] if not last else cps[h][0:64, :],
                            in1=rc, op=mult)
                    if p == 1:
                        # ctxT1 columns for this window are final -> out-proj
                        # parcels for the covered m-chunks can run (the last
                        # window's parcels are the short kp1 finishers)
                        for m in range(q0 // 128, (q0 + w) // 128):
                            for eo in range(2):
                                f = out_proj_parcel if jw < len(wins) - 1 else oproj_kp1
                                fillers.append(lambda m=m, eo=eo, f=f: f(m, eo))
            while fillers:
                fillers.popleft()()

    nc.compile()
    return nc


_NC_CACHE = {}


def _get_program():
    if "nc" not in _NC_CACHE:
        _NC_CACHE["nc"] = build_program()
    return _NC_CACHE["nc"]


def make_in_maps(x, Wq, bq, Wk, bk, Wv, bv, Wo, bo):
    x = np.asarray(x)
    xTs = [np.ascontiguousarray(x[b].T.astype(np.float16)) for b in range(B)]
    in_maps = []
    for c in range(N_CORES):
        b, hg = divmod(c, TP)
        sl = slice(hg * DQ, (hg + 1) * DQ)
        in_maps.append({
            "xT": xTs[b],
            "wqT": np.ascontiguousarray(np.asarray(Wq, np.float16)[sl, :].T),
            "wkT": np.ascontiguousarray(np.asarray(Wk, np.float16)[sl, :].T),
            "wvT": np.ascontiguousarray(np.asarray(Wv, np.float16)[sl, :].T),
            "woT": np.ascontiguousarray(np.asarray(Wo, np.float16)[:, sl].T),
            "bq_s": np.ascontiguousarray(np.asarray(bq, np.float32)[sl]),
            "bk_s": np.ascontiguousarray(np.asarray(bk, np.float32)[sl]),
        })
    return in_maps


def assemble_output(results, Wv_bias_term):
    out = np.empty((B, N, EMB), np.float32)
    for b in range(B):
        acc = results[b * TP]["out_part"].astype(np.float32)
        for g in range(1, TP):
            acc = acc + results[b * TP + g]["out_part"]
        out[b] = acc + Wv_bias_term
    return out


def kernel(x, Wq, bq, Wk, bk, Wv, bv, Wo, bo):
    nc = _get_program()
    in_maps = make_in_maps(x, Wq, bq, Wk, bk, Wv, bv, Wo, bo)
    res = run_bass_kernel_spmd(nc, in_maps, list(range(N_CORES)))
    bias_term = (np.asarray(bv, np.float32) @ np.asarray(Wo, np.float32).T
                 + np.asarray(bo, np.float32))
    return assemble_output(res.results, bias_term)


# revision 45
# speedup vs baseline: 1.0397x; 1.0125x over previous
"""Multi-head self-attention (b=2, n=2048, emb=1024, heads=16) on 8 trn2 cores.

Sharding: core c = (b, hg) with b = c // 4, hg = c % 4. Data parallel over
batch, tensor parallel over head-groups (4 heads / 256 emb-cols per core).
Each core computes Q/K/V projections for its heads, full attention for its
heads, and a partial output projection ctx_hg @ Wo[:, hg_slice].T of shape
[2048, 1024]. The host sums the 4 partials per batch (Megatron row-parallel
reduce done on host) and adds the rank-1 bias term bv @ Wo.T + bo.

Device layout notes:
- x^T [emb, n] fp16 (4MB = 32KB/partition) is loaded ONCE and stays resident
  in SBUF; both head-pairs' projections read it, so there is no mid-kernel
  x DMA traffic and no DMA wait can head-of-line-block the in-order PE queue.
- DMA issue is split across the Sync and GpSimd queues so sequencer DGE time
  doesn't serialize the startup (weights on Sync, x + output stores on
  GpSimd).
- Q^T, K^T are produced in [dq, n] layout (dq = head-major), V in natural
  [n, dv] layout augmented with a ones column per head -> the ctx matmul
  ctxT[65, nq] = V_aug^T @ E^T produces softmax row-sums in row 64 for free.
- Attention runs on 512-wide nq chunks: S matmuls, 1024-wide exps straight
  out of PSUM, and ctx matmuls all stream 512 columns, halving the PE
  instruction count (and its stall overhead) vs 256-wide chunks.
- Softmax normalization: row-sum staged to partition 0 (DVE), broadcast
  (GpSimd), reciprocal_approx_fast (single custom-DVE op, ~5x faster than
  InstReciprocal; denominators are strictly positive sums of exps so the
  approx's edge cases can't occur), then one fused multiply straight out of
  the ctx PSUM bank into ctxT fp16.
- Output projection accumulates both head-pair contributions into one PSUM
  group (2 matmuls back-to-back), needing a single PSUM->SBUF copy and no
  SBUF accumulator array.
- All matmuls run in float16 (1 cyc/col on PE; 10-bit mantissa keeps the
  overall error ~7e-4 scale-relative, validated vs fp32).
- q/k biases are added on-device (fused into the PSUM->SBUF copy);
  v/o biases are exactly the rank-1 host-side term above.
"""

import os
import sys

for _p in ("/opt/trn_rl_repo", "/root/.axon_site/_ro/trn_rl_repo"):
    if os.path.isdir(_p) and _p not in sys.path:
        sys.path.append(_p)

import numpy as np

import concourse.bass as bass  # noqa: F401  (engine types pulled via nc)
import concourse.mybir as mybir
import concourse.tile as tile
from concourse import bacc
from concourse.bass_utils import run_bass_kernel_spmd

B, N, EMB, HEADS, HD = 2, 2048, 1024, 16, 64
N_CORES = 8
TP = 4                      # head-group shards per batch
DQ = EMB // TP              # 256 emb-cols (4 heads) per core
SCALE = HD ** -0.5          # 0.125

F32 = mybir.dt.float32
F16 = mybir.dt.float16
FP = mybir.ActivationFunctionType

NQ = 512                    # nq chunk for projections / out-proj
NJ = N // NQ                # 4 nq chunks
NQA = 512                   # nq chunk for attention
NJA = N // NQA              # 4 attention nq chunks
NKC = 128                   # nk chunk (ctx contraction)
NT = N // NKC               # 16 nk chunks
KC = EMB // 128             # 8 e chunks
GK = 2                      # nk chunks per S-psum group (1024-wide exps)
NG = NT // GK               # 8 groups per (head, j)


def build_program():
    """Build + compile the single SPMD program all 8 cores run."""
    nc = bacc.Bacc("TRN2", target_bir_lowering=False, debug=False,
                   num_devices=N_CORES)

    xT = nc.dram_tensor("xT", [EMB, N], F16, kind="ExternalInput").ap()
    wqT = nc.dram_tensor("wqT", [EMB, DQ], F16, kind="ExternalInput").ap()
    wkT = nc.dram_tensor("wkT", [EMB, DQ], F16, kind="ExternalInput").ap()
    wvT = nc.dram_tensor("wvT", [EMB, DQ], F16, kind="ExternalInput").ap()
    woT = nc.dram_tensor("woT", [DQ, EMB], F16, kind="ExternalInput").ap()
    bqd = nc.dram_tensor("bq_s", [DQ], F32, kind="ExternalInput").ap()
    bkd = nc.dram_tensor("bk_s", [DQ], F32, kind="ExternalInput").ap()
    # fp16 partials: host sums 4 of them in fp32; quantization of the
    # partial (|.| ~ 1, ulp ~ 1e-3) adds ~1e-3 absmax-relative error --
    # well under the 2e-2 gate -- and halves the store traffic + tail.
    out_part = nc.dram_tensor("out_part", [N, EMB], F16,
                              kind="ExternalOutput").ap()

    with tile.TileContext(nc) as tc:
        with (
            tc.tile_pool(name="const", bufs=1) as const,
            tc.tile_pool(name="persist", bufs=1) as persist,
            tc.tile_pool(name="epool", bufs=3) as epool,
            tc.tile_pool(name="npool", bufs=2) as npool,
            tc.tile_pool(name="opool", bufs=4) as opool,
            # PSUM static budget (8 banks): pp 2 + s0 2 + s1 2 + c0 1 + c1 1
            tc.tile_pool(name="ppool", bufs=2, space="PSUM") as ppool,
            tc.tile_pool(name="spool", bufs=1, space="PSUM") as spool,
            tc.tile_pool(name="cpool", bufs=1, space="PSUM") as cpool,
        ):
            # ---- constants + resident x ----
            # first projection group needs wk chunks + x n-slice 0 only; those
            # DMAs go first, fine-grained, spread across the Sync / GpSimd /
            # Scalar queues so no single sequencer's DGE time serializes the
            # startup. x n-slice 0 is 8 small chunk DMAs (fast first arrival);
            # slices 1-3 ride 8 bigger strided slab DMAs.
            wq_sb = const.tile([128, KC, DQ], F16, tag="wq")
            wk_sb = const.tile([128, KC, DQ], F16, tag="wk")
            wv_sb = const.tile([128, KC, DQ], F16, tag="wv")
            x_sb = const.tile([128, KC, N], F16, tag="x")
            xTr = xT.rearrange("(k p) n -> k p n", p=128)
            # x chunks arrive in exactly the order the projection prefix
            # consumes them (n-major), spread over all three DMA-capable
            # queues (Sync/GpSimd/Scalar) so no single sequencer's ~0.6us
            # per-DMA DGE time serializes the startup: n-slice 0 is split
            # even/odd-k across GpSimd/Scalar for the fastest first arrival.
            bk_sb = const.tile([128, 2], F32, tag="bk")
            bq_sb = const.tile([128, 2], F32, tag="bq")
            nc.scalar.dma_start(out=bk_sb, in_=bkd.rearrange("(m p) -> p m", p=128))
            nc.scalar.dma_start(out=bq_sb, in_=bqd.rearrange("(m p) -> p m", p=128))
            for k in range(KC):
                nc.sync.dma_start(out=wk_sb[:, k, :], in_=wkT.rearrange(
                    "(k p) d -> k p d", p=128)[k])
                nc.gpsimd.dma_start(out=x_sb[:, k, 0:NQ], in_=xTr[k, :, 0:NQ])
                nc.scalar.dma_start(out=wv_sb[:, k, :], in_=wvT.rearrange(
                    "(k p) d -> k p d", p=128)[k])
            for k in range(KC):
                nc.sync.dma_start(out=wq_sb[:, k, :], in_=wqT.rearrange(
                    "(k p) d -> k p d", p=128)[k])
                nc.gpsimd.dma_start(out=x_sb[:, k, 2 * NQ:3 * NQ],
                                    in_=xTr[k, :, 2 * NQ:3 * NQ])
                nc.scalar.dma_start(out=x_sb[:, k, NQ:2 * NQ],
                                    in_=xTr[k, :, NQ:2 * NQ])
            for k in range(KC):
                nc.sync.dma_start(out=x_sb[:, k, 3 * NQ:N],
                                  in_=xTr[k, :, 3 * NQ:N])
            # wo is needed only by the out-projection (~100us in) — deferred
            # into the filler stream to keep startup queues clear
            wo_sb = const.tile([128, 2, EMB], F16, tag="wo")

            # ---- persistent activations ----
            qT = [persist.tile([128, N], F16, tag=f"qT{p}", name=f"qT{p}") for p in range(2)]
            kT = [persist.tile([128, N], F16, tag=f"kT{p}", name=f"kT{p}") for p in range(2)]
            ctxT = [persist.tile([128, N], F16, tag=f"ctxT{p}", name=f"ctxT{p}") for p in range(2)]
            # V for all 4 local heads: [nk-part, t, head*65 + (0:64 | ones)]
            v_all = persist.tile([128, NT, 4 * (HD + 1)], F16, tag="v")
            for h in range(4):
                nc.vector.memset(v_all[:, :, h * 65 + 64], 1.0)

            add, mult = mybir.AluOpType.add, mybir.AluOpType.mult

            # ---- projection building blocks ----
            # Each is one PSUM accumulation group on the double-buffered pp
            # tag, small enough to slot between attention groups.
            def kq_group(p, n, wsb, bsb, dst):
                ps = ppool.tile([128, NQ], F32, tag="pp", name="kqp")
                for k in range(KC):
                    nc.tensor.matmul(
                        ps, wsb[:, k, p * 128:(p + 1) * 128],
                        x_sb[:, k, n * NQ:(n + 1) * NQ],
                        start=(k == 0), stop=(k == KC - 1))
                nc.vector.tensor_tensor(
                    out=dst[p][:, n * NQ:(n + 1) * NQ], in0=ps,
                    in1=bsb[:, p:p + 1].broadcast_to([128, NQ]), op=add)

            def v_group(n, tl):
                # V for ALL 4 local heads at once (256-col moving wv): done
                # in the prefix so pair-1's share never burdens the p0
                # attention windows
                t = n * 4 + tl
                ps = ppool.tile([128, DQ], F32, tag="pp", name="vp")
                for k in range(KC):
                    nc.tensor.matmul(
                        ps, x_sb[:, k, t * 128:(t + 1) * 128],
                        wv_sb[:, k, :],
                        start=(k == 0), stop=(k == KC - 1))
                vv = v_all[:, t, :].rearrange("p (h c) -> p h c", c=65)
                nc.vector.tensor_copy(
                    out=vv[:, :, 0:64],
                    in_=ps.rearrange("p (h c) -> p h c", c=64))

            # Minimal cold prefix: the PE runs at ~half clock until ~3us of
            # continuous execution, so only the work attention j=0 strictly
            # needs before its first items runs up front — K(n=0), Q(n=0),
            # V(t=0,1). Everything else streams through the hot attention
            # windows as fillers (emission order constraints handled by the
            # per-window filler lists below).
            kq_group(0, 0, wk_sb, bk_sb, kT)
            kq_group(0, 0, wq_sb, bq_sb, qT)
            v_group(0, 0)
            v_group(0, 1)

            # ---- out-projection ----
            # one (m, eo) parcel: both head-pair passes accumulate into a
            # single pp-tag PSUM group, then one copy to SBUF + store.
            def out_proj_parcel(m, eo):
                po = ppool.tile([128, NQ], F32, tag="pp", name="po")
                for kp in range(2):
                    nc.tensor.matmul(
                        po, ctxT[kp][:, m * 128:(m + 1) * 128],
                        wo_sb[:, kp, eo * NQ:(eo + 1) * NQ],
                        start=(kp == 0), stop=(kp == 1))
                o = opool.tile([128, NQ], F16, tag="o", name="o")
                nc.vector.tensor_copy(o, po)
                nc.gpsimd.dma_start(
                    out=out_part[m * 128:(m + 1) * 128, eo * NQ:(eo + 1) * NQ],
                    in_=o)

            # The LAST window's m-chunks split the parcel: the ctxT0 half
            # runs during the window (ctxT0 is final since p0), leaving only
            # one matmul + add + store per parcel on the serial tail.
            o0s = {}

            def oproj_kp0(m, eo):
                po = ppool.tile([128, NQ], F32, tag="pp", name="po")
                nc.tensor.matmul(
                    po, ctxT[0][:, m * 128:(m + 1) * 128],
                    wo_sb[:, 0, eo * NQ:(eo + 1) * NQ], start=True, stop=True)
                o0 = opool.tile([128, NQ], F32, tag=f"o0_{m}_{eo}", name="o0")
                nc.vector.tensor_copy(o0, po)
                o0s[(m, eo)] = o0

            def oproj_kp1(m, eo):
                po = ppool.tile([128, NQ], F32, tag="pp", name="po")
                nc.tensor.matmul(
                    po, ctxT[1][:, m * 128:(m + 1) * 128],
                    wo_sb[:, 1, eo * NQ:(eo + 1) * NQ], start=True, stop=True)
                o = opool.tile([128, NQ], F16, tag="o", name="o")
                nc.vector.tensor_tensor(out=o, in0=o0s[(m, eo)], in1=po,
                                        op=add)
                nc.gpsimd.dma_start(
                    out=out_part[m * 128:(m + 1) * 128, eo * NQ:(eo + 1) * NQ],
                    in_=o)

            # ---- attention (per head-pair p, nq window of 512/256) ----
            # Software-pipelined: ctx matmuls for work item u are emitted
            # after the S/exp of item u+1, so PE always has ready work while
            # ACT streams 1024-wide exps; heads alternate as the natural PSUM
            # ping-pong for the S tiles. Projection/out-proj parcels are
            # popped between work items from per-window filler lists whose
            # ordering respects emission deadlines: K(n)/V(t)/Q(j) parcels
            # are always emitted before the first S/ctx that reads their
            # output. The final p=1 window is split into two 256-wide halves
            # so the serial tail covers half as many m-chunks.
            from collections import deque
            fillers = deque()

            def K0(n):
                return lambda: kq_group(0, n, wk_sb, bk_sb, kT)

            def Q0(n):
                return lambda: kq_group(0, n, wq_sb, bq_sb, qT)

            def K1(n):
                return lambda: kq_group(1, n, wk_sb, bk_sb, kT)

            def Q1(n):
                return lambda: kq_group(1, n, wq_sb, bq_sb, qT)

            def V(n, tl):
                return lambda: v_group(n, tl)

            wo_dma = lambda: nc.sync.dma_start(  # noqa: E731
                out=wo_sb, in_=woT.rearrange("(k p) e -> p k e", p=128))

            # per-window static filler lists (window key = (p, index))
            sched = {
                (0, 0): ([V(0, 2), V(0, 3), K0(1)]
                         + [V(1, tl) for tl in range(4)] + [K0(2)]
                         + [V(2, tl) for tl in range(4)] + [K0(3)]
                         + [V(3, tl) for tl in range(2)] + [Q0(1)]
                         + [V(3, tl) for tl in range(2, 4)]),
                (0, 1): [Q0(2), K1(0), Q1(0)],
                (0, 2): [Q0(3), K1(1), wo_dma],
                (0, 3): [K1(2)],
                (1, 0): [K1(3), Q1(1)],
                (1, 1): [Q1(2)],
                (1, 2): [Q1(3)],
                (1, 3): [lambda m=m, eo=eo: oproj_kp0(m, eo)
                         for m in range(14, 16) for eo in range(2)],
            }

            for p in range(2):
                if p == 0:
                    wins = [(jq * NQA, NQA) for jq in range(NJA)]
                else:
                    # final window split in two 256-halves: the serial tail
                    # (normalize chain + kp1 finishers + stores) covers only
                    # 2 m-chunks
                    wins = [(jq * NQA, NQA) for jq in range(NJA - 1)]
                    wins += [(N - NQA, NQA // 2), (N - NQA // 2, NQA // 2)]
                for jw, (q0, w) in enumerate(wins):
                    # projection fillers go to the FRONT so their DVE tails
                    # retire well before the next window's S reads them;
                    # out-proj parcels (appended at the previous window's
                    # end) drain after them.
                    statics = sched.get((p, jw), [])
                    fillers.extendleft(reversed(statics))
                    n_static = len(statics)
                    cps = [cpool.tile([HD + 1, w], F32, tag=f"c{h}",
                                      name=f"c{h}") for h in range(2)]

                    def s_mms(g, h, q0=q0, w=w):
                        lo = 64 * h
                        sp = spool.tile([128, GK, w], F32,
                                        tag=f"s{h}", name=f"s{h}")
                        for i, t in enumerate(g):
                            nc.tensor.matmul(
                                sp[:, i, :],
                                kT[p][lo:lo + 64, t * 128:(t + 1) * 128],
                                qT[p][lo:lo + 64, q0:q0 + w],
                                start=True, stop=True)
                        return sp

                    def exp_act(sp, g, h, w=w):
                        e = epool.tile([128, GK, w], F16,
                                       tag=f"e{h}", name=f"e{h}")
                        nc.scalar.activation(e, sp, FP.Exp, scale=SCALE)
                        return e

                    def ctx_mms(e, g, h):
                        hloc = 2 * p + h
                        for i, t in enumerate(g):
                            nc.tensor.matmul(
                                cps[h],
                                v_all[:, t, hloc * 65:(hloc + 1) * 65],
                                e[:, i, :],
                                start=(t == 0), stop=(t == NT - 1))

                    work = []
                    for gi in range(NG):
                        for h in range(2):
                            work.append((tuple(range(gi * GK, (gi + 1) * GK)), h))
                    n_pop = len(fillers)  # drain this window's list fully
                    popped = 0
                    prev = None
                    for wi, (g, h) in enumerate(work):
                        # order within an item: S(u) | filler | ctx(u-1) —
                        # the filler between S and ctx gives exp(u-1) the
                        # extra headroom to finish before the in-order PE
                        # queue reaches ctx(u-1), and keeps S(u+1) far enough
                        # behind exp(u-1)'s release of the S-psum tag.
                        sp = s_mms(g, h)
                        cur = (exp_act(sp, g, h), g, h)
                        # static projection fillers drain eagerly (one per
                        # item) so their DVE tails retire early; parcels are
                        # paced to finish ~4 items before the window's end so
                        # their PSUM->SBUF copies don't collide with the
                        # normalize chain on DVE
                        target = max(min(wi + 1, n_static),
                                     (wi + 1) * n_pop // max(1, len(work) - 4))
                        while fillers and popped < min(n_pop, target):
                            fillers.popleft()()
                            popped += 1
                        if prev is not None:
                            ctx_mms(*prev)
                        prev = cur
                    ctx_mms(*prev)
                    while fillers and popped < n_pop:
                        fillers.popleft()()
                        popped += 1

                    # normalize: ctx^T[0:64] * (1 / rowsum); rowsum in row 64.
                    # Two quick DVE copies (rowsum row -> partition 0 for
                    # partition_broadcast, ctx body -> SBUF) release the ctx
                    # PSUM bank ~1.5us after the last ctx matmul so the next
                    # window's ctx accumulation never stalls on it; the
                    # broadcast + reciprocal then run off the critical path.
                    last = (p == 1 and jw == len(wins) - 1)
                    css = [None, None]
                    for h in range(2):
                        rs = npool.tile([1, w], F32, tag="rs", name="rs")
                        nc.vector.tensor_copy(rs, cps[h][64:65, :])
                        if not last:
                            # mid-kernel: stage ctx to SBUF to release the c
                            # PSUM bank before the next window needs it
                            cs = npool.tile([64, w], F32, tag="cs", name="cs")
                            nc.vector.tensor_copy(cs, cps[h][0:64, :])
                            css[h] = cs
                        rb = npool.tile([64, w], F32, tag="rb", name="rb")
                        nc.gpsimd.partition_broadcast(rb, rs)
                        rc = npool.tile([64, w], F32, tag="rc", name="rc")
                        nc.vector.reciprocal_approx_fast(out=rc, in_=rb)
                        # last window: multiply straight out of PSUM — the
                        # shorter chain gates the tail's out-proj finishers
                        nc.vector.tensor_tensor(
                            out=ctxT[p][h * 64:(h + 1) * 64, q0:q0 + w],
                            in0=css[h] if not last else cps[h][0:64, :],
                            in1=rc, op=mult)
                    if p == 1:
                        # ctxT1 columns for this window are final -> out-proj
                        # parcels for the covered m-chunks can run (the last
                        # window's parcels are the short kp1 finishers)
                        for m in range(q0 // 128, (q0 + w) // 128):
                            for eo in range(2):
                                f = out_proj_parcel if jw < len(wins) - 1 else oproj_kp1
                                fillers.append(lambda m=m, eo=eo, f=f: f(m, eo))
            while fillers:
                fillers.popleft()()

    nc.compile()
    return nc


_NC_CACHE = {}


def _get_program():
    if "nc" not in _NC_CACHE:
        _NC_CACHE["nc"] = build_program()
    return _NC_CACHE["nc"]


def make_in_maps(x, Wq, bq, Wk, bk, Wv, bv, Wo, bo):
    x = np.asarray(x)
    xTs = [np.ascontiguousarray(x[b].T.astype(np.float16)) for b in range(B)]
    in_maps = []
    for c in range(N_CORES):
        b, hg = divmod(c, TP)
        sl = slice(hg * DQ, (hg + 1) * DQ)
        in_maps.append({
            "xT": xTs[b],
            "wqT": np.ascontiguousarray(np.asarray(Wq, np.float16)[sl, :].T),
            "wkT": np.ascontiguousarray(np.asarray(Wk, np.float16)[sl, :].T),
            "wvT": np.ascontiguousarray(np.asarray(Wv, np.float16)[sl, :].T),
            "woT": np.ascontiguousarray(np.asarray(Wo, np.float16)[:, sl].T),
            "bq_s": np.ascontiguousarray(np.asarray(bq, np.float32)[sl]),
            "bk_s": np.ascontiguousarray(np.asarray(bk, np.float32)[sl]),
        })
    return in_maps


def assemble_output(results, Wv_bias_term):
    out = np.empty((B, N, EMB), np.float32)
    for b in range(B):
        acc = results[b * TP]["out_part"].astype(np.float32)
        for g in range(1, TP):
            acc = acc + results[b * TP + g]["out_part"]
        out[b] = acc + Wv_bias_term
    return out


def kernel(x, Wq, bq, Wk, bk, Wv, bv, Wo, bo):
    nc = _get_program()
    in_maps = make_in_maps(x, Wq, bq, Wk, bk, Wv, bv, Wo, bo)
    res = run_bass_kernel_spmd(nc, in_maps, list(range(N_CORES)))
    bias_term = (np.asarray(bv, np.float32) @ np.asarray(Wo, np.float32).T
                 + np.asarray(bo, np.float32))
    return assemble_output(res.results, bias_term)
